# revision 5
# baseline (speedup 1.0000x reference)
"""Causal self-attention on 8 Trainium2 NeuronCores, tensor-parallel over heads.

Problem: B=2, T=2048, C=1024, H=16 heads (dk=64). Each core owns 2 heads.

v2 design (transpose-free attention, bf16 matmuls):
  1. Q^T,K^T feature-major ([128 feat, T], head0 rows 0:64, head1 64:128);
     V time-major ([T, d]) computed directly as x-tile.T @ w_v, stored with
     an appended ones column (Z trick). V's bias is NOT applied on device:
     softmax weights sum to 1, so it commutes through attention and the
     host folds b_v @ w_out into the final bias add.
  2. Scores computed k-major: S^T[k,q] = matmul(lhsT=K^T_j, rhs=Q^T_i) so
     P^T = exp(S^T/8) comes straight out of the ACT exp with no PE
     transposes. Causal diag masked via identity@maskT matmul add.
  3. PV q-major: attn[q, 0:64|Z] = sum_j P^T_j.T @ V_aug_j; col 64 is the
     softmax denominator Z (ones column of V_aug). Normalize with
     reciprocal+tensor_scalar on DVE.
  4. attn^T via one PE bf16 transpose per q-tile; y = attnT.T @ w_out
     (q-major, N=512), stored f32 directly from PSUM to DRAM.
Host: shards w_qkv/w_out by head pair, replicates x^T in bf16, sums the 8
f32 partial outputs + b_out.
"""

import numpy as np
from contextlib import ExitStack

import concourse.bass as bass
import concourse.tile as tile
from concourse import bacc, mybir
from concourse.masks import make_identity

F32 = mybir.dt.float32
BF16 = mybir.dt.bfloat16
FP8 = mybir.dt.float8e4
AF = mybir.ActivationFunctionType
WSCALE = 32.0           # host multiplies w_qkv/b_qkv by this for fp8 range

C = 1024
DK = 64
HP = 2                  # heads per core
N_CORES = 8
KT = C // 128           # k-tiles over the C contraction
MASK_VAL = -3e10


def _emit(ctx: ExitStack, tc: tile.TileContext, aps: dict, B: int, T: int,
          reps: int = 1):
    nc = tc.nc
    xT, wqk, wv, bqk, wo, y = (
        aps[k] for k in ("xT", "wqk", "wv", "bqk", "wo", "y"))
    NT = T // 128        # q tiles per batch
    NB = T // 512

    consts = ctx.enter_context(tc.tile_pool(name="consts", bufs=1))
    xpool = ctx.enter_context(tc.tile_pool(name="x", bufs=1))
    qkvp = ctx.enter_context(tc.tile_pool(name="qkT", bufs=2))
    vpool = ctx.enter_context(tc.tile_pool(name="v", bufs=2))
    ppool = ctx.enter_context(tc.tile_pool(name="p", bufs=18))
    small = ctx.enter_context(tc.tile_pool(name="small", bufs=4))
    attnp = ctx.enter_context(tc.tile_pool(name="attn", bufs=2))
    outp = ctx.enter_context(tc.tile_pool(name="out", bufs=2))
    # PSUM layout (bank = 2KB/partition; matmul targets must not straddle):
    #  s: 2 bufs x 2 banks | mm: 2 x 1 bank (qkv-proj, V, transpose, out-proj)
    #  pv: 2 x 1 bank  -> 8 banks total
    psum_s = ctx.enter_context(tc.tile_pool(name="psum_s", bufs=2, space="PSUM"))
    psum_mm = ctx.enter_context(tc.tile_pool(name="psum_mm", bufs=2, space="PSUM"))
    psum_pv = ctx.enter_context(tc.tile_pool(name="psum_pv", bufs=2, space="PSUM"))

    ident_bf = consts.tile([128, 128], BF16)
    make_identity(nc, ident_bf)
    ident_f32 = consts.tile([128, 128], F32)
    make_identity(nc, ident_f32)
    # S^T mask: keep (0) where k <= q, MASK_VAL where k > q (k = partition)
    maskT = consts.tile([128, 128], BF16)
    nc.gpsimd.memset(maskT, 0.0)
    # out[k, q] = (-k + q) >= 0 ? 0 : MASK_VAL  (keep k <= q)
    nc.gpsimd.affine_select(
        out=maskT, in_=maskT, compare_op=mybir.AluOpType.is_ge,
        fill=MASK_VAL, base=0, pattern=[[1, 128]], channel_multiplier=-1,
    )
    wqk_sb = consts.tile([128, KT, 2 * 128], BF16)
    nc.sync.dma_start(out=wqk_sb, in_=wqk.rearrange("(kt p) m -> p kt m", p=128))
    wv_sb = consts.tile([128, KT, 128], BF16)
    nc.sync.dma_start(out=wv_sb, in_=wv.rearrange("(kt p) m -> p kt m", p=128))
    bqk_sb = consts.tile([128, 2], F32)
    nc.sync.dma_start(out=bqk_sb, in_=bqk)
    wo_sb = consts.tile([128, C], BF16)
    nc.sync.dma_start(out=wo_sb, in_=wo)

    for _rep in range(reps):
        x_kt = []
        for kt in range(KT):
            t_ = xpool.tile([128, B * T], BF16, tag=f"x{kt}")
            nc.sync.dma_start(out=t_, in_=xT[kt * 128:(kt + 1) * 128, :])
            x_kt.append(t_)

        for b in range(B):
            # ---- projections ----
            qkT = qkvp.tile([128, 2, T], BF16, tag="qkT")
            for m in range(2):          # 0 = Q^T, 1 = K^T
                for nb in range(NB):
                    ps = psum_mm.tile([128, 512], F32, tag="mm")
                    for kt in range(KT):
                        nc.tensor.matmul(
                            ps,
                            lhsT=wqk_sb[:, kt, m * 128:(m + 1) * 128],
                            rhs=x_kt[kt][:, b * T + nb * 512:b * T + (nb + 1) * 512],
                            start=(kt == 0),
                            stop=(kt == KT - 1),
                        )
                    nc.vector.tensor_scalar_add(
                        qkT[:, m, nb * 512:(nb + 1) * 512],
                        ps, bqk_sb[:, m:m + 1],
                    )
            v_sb = vpool.tile([128, NT, HP, DK + 1], BF16, tag="v")
            nc.gpsimd.memset(v_sb, 1.0)     # ones column for Z; rest overwritten
            for t in range(NT):
                ps_ = psum_mm.tile([128, 512], F32, tag="mm")
                pv_ = ps_[:, 0:128]
                for kt in range(KT):
                    nc.tensor.matmul(
                        pv_,
                        lhsT=x_kt[kt][:, b * T + t * 128:b * T + (t + 1) * 128],
                        rhs=wv_sb[:, kt, :],
                        start=(kt == 0),
                        stop=(kt == KT - 1),
                    )
                nc.vector.tensor_copy(
                    v_sb[:, t, :, 0:DK],
                    pv_.rearrange("p (h d) -> p h d", h=HP),
                )

            # ---- attention: chunk-outer S^T (wide-q), per-tile PV ----
            # chunk c covers q in [512c, 512c+512); S^T_j computed once per
            # (c, j<=4c+3) at exact causal width; P_(c,j) tiles live for the
            # whole chunk. q-tile tails are software-pipelined via `pend`.
            pend = None  # (attn_sb, i) awaiting transpose/out-proj
            for c in range(NB):
                p_c = []        # p_c[j] = (tile, q0) ; cols map q = q0 + col
                for j in range(4 * c + 4):
                    q0 = max(j * 128, c * 512)
                    W = (c + 1) * 512 - q0
                    diag = j * 128 >= c * 512     # j in this chunk's group
                    ps = psum_s.tile([128, 2, 512], F32, tag="s")
                    for h in range(HP):
                        hb = h * DK
                        nc.tensor.matmul(
                            ps[:, h, 0:W],
                            lhsT=qkT[hb:hb + DK, 1, j * 128:(j + 1) * 128],
                            rhs=qkT[hb:hb + DK, 0, q0:q0 + W],
                            start=True,
                            stop=True,
                        )
                    pb = ppool.tile([128, 2, 512], BF16, tag="p")
                    nc.scalar.activation(
                        out=pb[:, :, 0:W], in_=ps[:, :, 0:W],
                        func=AF.Exp, bias=0.0, scale=float(DK) ** -0.5,
                    )
                    if diag:
                        # zero the upper triangle of the diag block (k > q)
                        for h in range(HP):
                            nc.gpsimd.affine_select(
                                out=pb[:, h, 0:128], in_=pb[:, h, 0:128],
                                compare_op=mybir.AluOpType.is_ge,
                                fill=0.0, base=0, pattern=[[1, 128]],
                                channel_multiplier=-1,
                            )
                    p_c.append((pb, q0))

                for i in range(4 * c, 4 * c + 4):
                    pv = psum_pv.tile([128, 2, 128], F32, tag="pv")
                    for h in range(HP):
                        for j in range(i + 1):
                            pb, q0 = p_c[j]
                            c0 = i * 128 - q0
                            nc.tensor.matmul(
                                pv[:, h, 0:DK + 1],
                                lhsT=pb[:, h, c0:c0 + 128],
                                rhs=v_sb[:, j, h, :],
                                start=(j == 0),
                                stop=(j == i),
                            )
                    zr = small.tile([128, 2], F32, tag="zr")
                    nc.vector.reciprocal(zr, pv[:, :, DK])
                    attn = attnp.tile([128, 128], F32, tag="attn")
                    for h in range(HP):
                        nc.vector.tensor_scalar_mul(
                            attn[:, h * DK:(h + 1) * DK],
                            pv[:, h, 0:DK], zr[:, h:h + 1],
                        )
                    if pend is not None:
                        _emit_tail(nc, tc, aps, b, T, pend, ident_f32, wo_sb,
                                   attnp, psum_mm, outp)
                    pend = (attn, i)
            _emit_tail(nc, tc, aps, b, T, pend, ident_f32, wo_sb,
                       attnp, psum_mm, outp)


def _emit_tail(nc, tc, aps, b, T, pend, ident_f32, wo_sb, attnp, psum_mm,
               outp):
    attn, i = pend
    y = aps["y"]
    pt = psum_mm.tile([128, 512], F32, tag="mm")
    nc.tensor.transpose(pt[:, 0:128], in_=attn, identity=ident_f32)
    attnT = attnp.tile([128, 128], BF16, tag="attnT")
    nc.vector.tensor_copy(attnT, pt[:, 0:128])
    o_sb = outp.tile([128, C], BF16, tag="o")
    for half in range(2):
        op = psum_mm.tile([128, 512], F32, tag="mm")
        nc.tensor.matmul(
            op, lhsT=attnT, rhs=wo_sb[:, half * 512:(half + 1) * 512],
            start=True, stop=True,
        )
        nc.vector.tensor_copy(o_sb[:, half * 512:(half + 1) * 512], op)
    nc.sync.dma_start(
        out=y[b * T + i * 128:b * T + (i + 1) * 128, :], in_=o_sb,
    )


def build(B: int = 2, T: int = 2048, reps: int = 1):
    nc = bacc.Bacc("TRN2", target_bir_lowering=False, debug=False)
    BT = B * T
    aps = {
        "xT": nc.dram_tensor("xT", [C, BT], BF16, kind="ExternalInput").ap(),
        "wqk": nc.dram_tensor("wqk", [C, 256], BF16, kind="ExternalInput").ap(),
        "wv": nc.dram_tensor("wv", [C, 128], BF16, kind="ExternalInput").ap(),
        "bqk": nc.dram_tensor("bqk", [128, 2], F32, kind="ExternalInput").ap(),
        "wo": nc.dram_tensor("wo", [HP * DK, C], BF16, kind="ExternalInput").ap(),
        "y": nc.dram_tensor("y", [BT, C], BF16, kind="ExternalOutput").ap(),
    }
    with tile.TileContext(nc) as tc:
        with ExitStack() as ctx:
            _emit(ctx, tc, aps, B, T, reps=reps)
    nc.compile()
    return nc


def shard_inputs(x, w_qkv, b_qkv, w_out):
    """Host-side sharding: returns per-core input maps."""
    import ml_dtypes
    bf16 = ml_dtypes.bfloat16
    fp8 = mybir.dt.np(FP8)
    x = np.asarray(x, np.float32)
    w_qkv = np.asarray(w_qkv, np.float32)
    b_qkv = np.asarray(b_qkv, np.float32)
    w_out = np.asarray(w_out, np.float32)
    B, T, C_ = x.shape
    xT = np.ascontiguousarray(x.reshape(B * T, C_).T.astype(bf16))
    in_maps = []
    for g in range(N_CORES):
        h0, h1 = 2 * g, 2 * g + 1
        qcols = np.concatenate([np.arange(h0 * 192, h0 * 192 + 64),
                                np.arange(h1 * 192, h1 * 192 + 64)])
        kcols = qcols + 64
        vcols = qcols + 128
        in_maps.append({
            "xT": xT,
            "wqk": np.ascontiguousarray(
                w_qkv[:, np.concatenate([qcols, kcols])].astype(bf16)),
            "wv": np.ascontiguousarray(w_qkv[:, vcols].astype(bf16)),
            "bqk": np.ascontiguousarray(
                np.stack([b_qkv[qcols], b_qkv[kcols]], axis=1)),
            "wo": np.ascontiguousarray(
                w_out[g * 128:(g + 1) * 128, :].astype(bf16)),
        })
    return in_maps


_built = {}


def _get_nc(B, T, reps=1):
    if (B, T, reps) not in _built:
        _built[(B, T, reps)] = build(B, T, reps)
    return _built[(B, T, reps)]


def run(x, w_qkv, b_qkv, w_out, b_out, trace=False, trace_kwargs=None):
    from concourse.bass_utils import run_bass_kernel_spmd

    B, T, C_ = np.asarray(x).shape
    in_maps = shard_inputs(x, w_qkv, b_qkv, w_out)
    nc = _get_nc(B, T)
    res = run_bass_kernel_spmd(
        nc, in_maps, list(range(N_CORES)), trace=trace, **(trace_kwargs or {})
    )
    y = np.zeros((B * T, C_), np.float32)
    for g in range(N_CORES):
        y += np.asarray(res.results[g]["y"], np.float32)
    # V-bias commutes through softmax (weights sum to 1): add b_v @ w_out here
    b_qkv = np.asarray(b_qkv, np.float32)
    H = C_ // DK
    bv_full = np.concatenate(
        [b_qkv[h * 3 * DK + 2 * DK:h * 3 * DK + 3 * DK] for h in range(H)])
    y += np.asarray(b_out, np.float32) + bv_full @ np.asarray(w_out, np.float32)
    return y.reshape(B, T, C_), res


def kernel(x, w_qkv, b_qkv, w_out, b_out):
    y, _ = run(x, w_qkv, b_qkv, w_out, b_out)
    return y
